# revision 7
# baseline (speedup 1.0000x reference)
"""Trainium2 Bass kernel for nn_Event_Critic_Net (dual-branch GAT critic).

Math: the reference only reads the GAT output at the LAST node of each
graph (graphs are 32 contiguous nodes), so only edges whose dst is a
graph's last node contribute.  For those edges the softmax-weighted
aggregation commutes with the linear projection W:

    out_g = sigmoid( (sum_n alpha[n] * x[n,:]) @ W + bias )
    alpha[n] = cnt[n]*exp(e[n]) / (sum_n cnt[n]*exp(e[n]) + 1e-16)
    e[n] = leaky_relu(x[n]. w_src + x[last(g)]. w_dst),  w_* = W @ att_*

cnt[n] = number of edges (n -> last(g(n))).  Graph-structure prep
(edge counts, tiling, weight replication) happens on host; all FLOPs
on device.  Sharding: graphs are data-parallel across the 8 cores
(core c owns graphs [c*512, (c+1)*512) == nodes [c*16384, (c+1)*16384)).
"""

import numpy as np
from contextlib import ExitStack

NC = 8            # cores
N = 131072        # nodes total
G = 4096          # graphs
NPG = 32          # nodes per graph
S = 64            # state size
H = 128           # hidden size
NPC = N // NC     # 16384 nodes per core
GPC = G // NC     # 512 graphs per core
T = NPC // 128    # 128 node-tiles per core
SA = S + 1        # x augmented with a ones column (-> denominator)
CH = 16           # node-tiles per a_src chunk
NCHUNK = T // CH  # 8 chunks

_CACHE = {}


def _build_module():
    import concourse.tile as tile
    from concourse import bacc, mybir
    from concourse.alu_op_type import AluOpType as Alu

    f32 = mybir.dt.float32
    Act = mybir.ActivationFunctionType
    AxX = mybir.AxisListType.X

    nc = bacc.Bacc("TRN2", target_bir_lowering=False, debug=False,
                   num_devices=NC)

    dram = {}

    def din(name, shape):
        dram[name] = nc.dram_tensor(name, shape, f32, kind="ExternalInput")

    for p in ("u", "d"):
        din(f"{p}_xab", [128, T * SA])
        din(f"{p}_cnt", [128, T])
        din(f"{p}_xlast", [128, 4 * S])
        din(f"{p}_wsrc", [128, CH * SA])
        din(f"{p}_wdst", [128, 4 * S])
        din(f"{p}_bias", [128, 1])
        din(f"{p}_W", [S, H])
    din("Qm", [4, 128])
    din("Bm", [128, 4])
    din("ones64", [1, S])
    din("ident", [128, 128])
    din("mlpW", [H, 1])
    din("mlpb", [1, 1])
    din("eps", [1, 1])
    out_dram = nc.dram_tensor("out", [1, GPC], f32, kind="ExternalOutput")

    with tile.TileContext(nc) as tc, ExitStack() as ctx:
        const = ctx.enter_context(tc.tile_pool(name="const", bufs=1))
        xp = ctx.enter_context(tc.tile_pool(name="xp", bufs=2))
        wk = ctx.enter_context(tc.tile_pool(name="wk", bufs=2))
        ps1 = ctx.enter_context(tc.tile_pool(name="ps1", bufs=1, space="PSUM"))
        ps2 = ctx.enter_context(tc.tile_pool(name="ps2", bufs=2, space="PSUM"))

        def cload(name, shape):
            t = const.tile(shape, f32, tag=name)
            nc.sync.dma_start(t[:], dram[name].ap())
            return t

        Qm = cload("Qm", [4, 128])
        Bm = cload("Bm", [128, 4])
        ones64 = cload("ones64", [1, S])
        ident = cload("ident", [128, 128])
        mlpW = cload("mlpW", [H, 1])
        mlpb = cload("mlpb", [1, 1])
        eps = cload("eps", [1, 1])

        sig = {}
        for p in ("u", "d"):
            wsrc = cload(f"{p}_wsrc", [128, CH * SA])
            wdst = cload(f"{p}_wdst", [128, 4 * S])
            Wb = cload(f"{p}_W", [S, H])
            bias = cload(f"{p}_bias", [128, 1])

            # ---- big x load, chunked for DMA/compute overlap ----
            x = xp.tile([128, T * SA], f32, tag="x")
            for c in range(NCHUNK):
                sl = slice(c * CH * SA, (c + 1) * CH * SA)
                nc.sync.dma_start(x[:, sl], dram[f"{p}_xab"].ap()[:, sl])
            cnt = wk.tile([128, T], f32, tag="cnt")
            nc.sync.dma_start(cnt[:], dram[f"{p}_cnt"].ap())
            xl = wk.tile([128, 4 * S], f32, tag="xl")
            nc.sync.dma_start(xl[:], dram[f"{p}_xlast"].ap())

            # ---- a_dst at last nodes: mult+reduce, transpose, broadcast ----
            tmp4 = wk.tile([128, 4 * S], f32, tag="tmp4")
            nc.vector.tensor_tensor(tmp4[:], xl[:], wdst[:], op=Alu.mult)
            adst = wk.tile([128, 4], f32, tag="adst")
            nc.vector.tensor_reduce(
                adst[:], tmp4[:].rearrange("p (j s) -> p j s", s=S),
                axis=AxX, op=Alu.add)
            tp = ps1.tile([4, 128], f32, tag="tp")
            nc.tensor.transpose(tp[:], adst[:], ident[:])
            adT = wk.tile([4, 128], f32, tag="adT")
            nc.scalar.copy(adT[:], tp[:])
            adbc = ps1.tile([128, T], f32, tag="adbc")
            nc.tensor.matmul(adbc[:], Qm[:], adT[:], start=True, stop=True)

            # ---- a_src for all nodes: chunked mult + segmented reduce ----
            asrc = wk.tile([128, T], f32, tag="asrc")
            for c in range(NCHUNK):
                tmp = wk.tile([128, CH * SA], f32, tag="tmp")
                nc.vector.tensor_tensor(
                    tmp[:], x[:, c * CH * SA:(c + 1) * CH * SA], wsrc[:],
                    op=Alu.mult)
                nc.vector.tensor_reduce(
                    asrc[:, c * CH:(c + 1) * CH],
                    tmp[:].rearrange("p (i s) -> p i s", s=SA),
                    axis=AxX, op=Alu.add)

            # ---- P = cnt * exp(leaky_relu(a_src + a_dst)) ----
            z = wk.tile([128, T], f32, tag="z")
            nc.vector.tensor_tensor(z[:], asrc[:], adbc[:], op=Alu.add)
            e = wk.tile([128, T], f32, tag="e")
            nc.vector.scalar_tensor_tensor(
                e[:], z[:], 0.2, z[:], op0=Alu.mult, op1=Alu.max)
            ex = wk.tile([128, T], f32, tag="ex")
            nc.scalar.activation(ex[:], e[:], Act.Exp)
            P = wk.tile([128, T], f32, tag="P")
            nc.vector.tensor_tensor(P[:], ex[:], cnt[:], op=Alu.mult)

            # ---- M[p, 4i+j] = P[p, i] * B[p, j] (block-diag weights) ----
            M = wk.tile([128, 4 * T], f32, tag="M")
            Mv = M[:].rearrange("p (i j) -> p i j", j=4)
            for j in range(4):
                nc.vector.tensor_scalar(
                    Mv[:, :, j], P[:], Bm[:, j:j + 1], None, op0=Alu.mult)

            # ---- y^T aggregation: 128 small matmuls ----
            ynT = ps2.tile([128, 4 * T], f32, tag="ynT")
            for i in range(T):
                nc.tensor.matmul(
                    ynT[0:SA, 4 * i:4 * (i + 1)],
                    x[:, SA * i:SA * (i + 1)],
                    M[:, 4 * i:4 * (i + 1)],
                    start=True, stop=True)

            # ---- normalize by denominator (row 64 of y^T) ----
            ysb = wk.tile([SA, GPC], f32, tag="ysb")
            nc.scalar.copy(ysb[:], ynT[0:SA, :])
            dn = wk.tile([1, GPC], f32, tag="dn")
            nc.scalar.activation(dn[:], ysb[S:SA, :], Act.Identity, bias=eps[:])
            rp = wk.tile([1, GPC], f32, tag="rp")
            nc.vector.reciprocal(rp[:], dn[:])
            rbc = ps1.tile([S, GPC], f32, tag="rbc")
            nc.tensor.matmul(rbc[:], ones64[:], rp[:], start=True, stop=True)
            ynrm = wk.tile([S, GPC], f32, tag="ynrm")
            nc.vector.tensor_tensor(ynrm[:], ysb[0:S, :], rbc[:], op=Alu.mult)

            # ---- project + bias + sigmoid ----
            hT = ps1.tile([H, GPC], f32, tag="hT")
            nc.tensor.matmul(hT[:], Wb[:], ynrm[:], start=True, stop=True)
            sg = wk.tile([H, GPC], f32, tag="sig")
            nc.scalar.activation(sg[:], hT[:], Act.Sigmoid, bias=bias[:])
            sig[p] = sg

        # ---- combine branches + MLP head ----
        prod = wk.tile([H, GPC], f32, tag="prod")
        nc.vector.tensor_tensor(prod[:], sig["u"][:], sig["d"][:], op=Alu.mult)
        o_ps = ps1.tile([1, GPC], f32, tag="o_ps")
        nc.tensor.matmul(o_ps[:], mlpW[:], prod[:], start=True, stop=True)
        o_sb = wk.tile([1, GPC], f32, tag="o_sb")
        nc.scalar.activation(o_sb[:], o_ps[:], Act.Identity, bias=mlpb[:])
        nc.sync.dma_start(out_dram.ap(), o_sb[:])

    nc.compile()
    return nc


def _get_module():
    if "nc" not in _CACHE:
        _CACHE["nc"] = _build_module()
    return _CACHE["nc"]


def _prep_branch(x, ei, W, att_src, att_dst, bias):
    """Host-side sharding + graph-format prep for one branch."""
    x = np.asarray(x, np.float32)
    src = np.asarray(ei[0]).astype(np.int64)
    dst = np.asarray(ei[1]).astype(np.int64)
    W = np.asarray(W, np.float32)
    w_src = (W @ np.asarray(att_src, np.float32)).astype(np.float32)
    w_dst = (W @ np.asarray(att_dst, np.float32)).astype(np.float32)

    valid = (dst % NPG) == (NPG - 1)
    cnt = np.bincount(src[valid], minlength=N).astype(np.float32)

    per_core = []
    for c in range(NC):
        xs = x[c * NPC:(c + 1) * NPC]
        xab = np.ones((T, 128, SA), np.float32)
        xab[:, :, :S] = xs.reshape(T, 128, S)
        xab = np.ascontiguousarray(
            xab.transpose(1, 0, 2).reshape(128, T * SA))
        cnt_t = np.ascontiguousarray(
            cnt[c * NPC:(c + 1) * NPC].reshape(T, 128).T)
        xlast = np.ascontiguousarray(xs[NPG - 1::NPG].reshape(128, 4 * S))
        per_core.append({"xab": xab, "cnt": cnt_t, "xlast": xlast})

    wsrc_rep = np.zeros((128, CH * SA), np.float32)
    wsrc_rep.reshape(128, CH, SA)[:, :, :S] = w_src
    wdst_rep = np.broadcast_to(w_dst, (128, 4, S)).reshape(128, 4 * S).copy()
    shared = {
        "wsrc": wsrc_rep,
        "wdst": wdst_rep,
        "W": W,
        "bias": np.asarray(bias, np.float32).reshape(H, 1),
    }
    return per_core, shared


def _build_in_maps(inputs):
    pcs = {}
    shareds = {}
    pcs["u"], shareds["u"] = _prep_branch(
        inputs["up_x"], inputs["up_edge_index"], inputs["up_W"],
        inputs["up_att_src"], inputs["up_att_dst"], inputs["up_bias"])
    pcs["d"], shareds["d"] = _prep_branch(
        inputs["down_x"], inputs["down_edge_index"], inputs["down_W"],
        inputs["down_att_src"], inputs["down_att_dst"], inputs["down_bias"])

    pp = np.arange(128)
    Qm = np.zeros((4, 128), np.float32)
    Qm[pp // 32, pp] = 1.0
    Bm = np.zeros((128, 4), np.float32)
    Bm[pp, pp // 32] = 1.0

    common = {
        "Qm": Qm,
        "Bm": Bm,
        "ones64": np.ones((1, S), np.float32),
        "ident": np.eye(128, dtype=np.float32),
        "mlpW": np.asarray(inputs["mlp_W"], np.float32).reshape(H, 1),
        "mlpb": np.asarray(inputs["mlp_b"], np.float32).reshape(1, 1),
        "eps": np.full((1, 1), 1e-16, np.float32),
    }
    for p in ("u", "d"):
        for k, v in shareds[p].items():
            common[f"{p}_{k}"] = v

    in_maps = []
    for c in range(NC):
        m = dict(common)
        for p in ("u", "d"):
            for k, v in pcs[p][c].items():
                m[f"{p}_{k}"] = v
        in_maps.append(m)
    return in_maps


def kernel(**inputs):
    from concourse.bass_utils import run_bass_kernel_spmd

    nc = _get_module()
    in_maps = _build_in_maps(inputs)
    res = run_bass_kernel_spmd(nc, in_maps, core_ids=list(range(NC)))
    out = np.concatenate(
        [np.asarray(r["out"], np.float32).reshape(GPC) for r in res.results])
    return out.reshape(G, 1)


# revision 8
# speedup vs baseline: 1.6484x; 1.6484x over previous
"""Trainium2 Bass kernel for nn_Event_Critic_Net (dual-branch GAT critic).

Math: the reference only reads the GAT output at the LAST node of each
graph (graphs are 32 contiguous nodes), so only edges whose dst is a
graph's last node contribute.  For those edges the softmax-weighted
aggregation commutes with the linear projection W:

    out_g = sigmoid( (sum_n alpha[n] * x[n,:]) @ W + bias )
    alpha[n] = cnt[n]*exp(e[n]) / (sum_n cnt[n]*exp(e[n]) + 1e-16)
    e[n] = leaky_relu(x[n]. w_src + x[last(g)]. w_dst),  w_* = W @ att_*

cnt[n] = number of edges (n -> last(g(n))).  Graph-structure prep
(edge counts, tiling, weight replication) happens on host; all FLOPs
on device.  Sharding: graphs are data-parallel across the 8 cores
(core c owns graphs [c*512, (c+1)*512) == nodes [c*16384, (c+1)*16384)).

Data path is bf16 (PSUM accumulation fp32); softmax/normalization
scalars stay fp32.
"""

import numpy as np
from contextlib import ExitStack

NC = 8            # cores
N = 131072        # nodes total
G = 4096          # graphs
NPG = 32          # nodes per graph
S = 64            # state size
H = 128           # hidden size
NPC = N // NC     # 16384 nodes per core
GPC = G // NC     # 512 graphs per core
T = NPC // 128    # 128 node-tiles per core
SA = 66           # x columns: 64 features | ones | zero pad (4B align)
CH = 16           # node-tiles per a_src chunk
NCHUNK = T // CH  # 8 chunks

_CACHE = {}


def _build_module():
    import concourse.tile as tile
    from concourse import bacc, mybir
    from concourse.alu_op_type import AluOpType as Alu

    f32 = mybir.dt.float32
    bf16 = mybir.dt.bfloat16
    Act = mybir.ActivationFunctionType
    AxX = mybir.AxisListType.X

    nc = bacc.Bacc("TRN2", target_bir_lowering=False, debug=False,
                   num_devices=NC)

    dram = {}

    def din(name, shape, dt=f32):
        dram[name] = nc.dram_tensor(name, shape, dt, kind="ExternalInput")

    for p in ("u", "d"):
        din(f"{p}_xab", [128, T * SA], bf16)
        din(f"{p}_cnt", [128, T])
        din(f"{p}_xlast", [128, 4 * S], bf16)
        din(f"{p}_wsrc", [128, CH * SA], bf16)
        din(f"{p}_wdst", [128, 4 * S], bf16)
        din(f"{p}_bias", [128, 1])
        din(f"{p}_W", [S, H], bf16)
    din("Qm", [4, 128], bf16)
    din("Bm", [128, 4])
    din("ones64", [1, S])
    din("ident", [128, 128])
    din("mlpW", [H, 1], bf16)
    din("mlpb", [1, 1])
    din("eps", [1, 1])
    out_dram = nc.dram_tensor("out", [1, GPC], f32, kind="ExternalOutput")

    with tile.TileContext(nc) as tc, ExitStack() as ctx:
        const = ctx.enter_context(tc.tile_pool(name="const", bufs=1))
        xp = ctx.enter_context(tc.tile_pool(name="xp", bufs=2))
        wk = ctx.enter_context(tc.tile_pool(name="wk", bufs=2))
        ps1 = ctx.enter_context(tc.tile_pool(name="ps1", bufs=1, space="PSUM"))
        ps2 = ctx.enter_context(tc.tile_pool(name="ps2", bufs=2, space="PSUM"))

        def cload(name, shape, dt=f32):
            t = const.tile(shape, dt, tag=name)
            nc.sync.dma_start(t[:], dram[name].ap())
            return t

        Qm = cload("Qm", [4, 128], bf16)
        Bm = cload("Bm", [128, 4])
        ones64 = cload("ones64", [1, S])
        ident = cload("ident", [128, 128])
        mlpW = cload("mlpW", [H, 1], bf16)
        mlpb = cload("mlpb", [1, 1])
        eps = cload("eps", [1, 1])

        sig = {}
        for p in ("u", "d"):
            wsrc = cload(f"{p}_wsrc", [128, CH * SA], bf16)
            wdst = cload(f"{p}_wdst", [128, 4 * S], bf16)
            Wb = cload(f"{p}_W", [S, H], bf16)
            bias = cload(f"{p}_bias", [128, 1])

            # ---- big x load, chunked for DMA/compute overlap ----
            x = xp.tile([128, T * SA], bf16, tag="x")
            for c in range(NCHUNK):
                sl = slice(c * CH * SA, (c + 1) * CH * SA)
                nc.sync.dma_start(x[:, sl], dram[f"{p}_xab"].ap()[:, sl])
            cnt = wk.tile([128, T], f32, tag="cnt")
            nc.sync.dma_start(cnt[:], dram[f"{p}_cnt"].ap())
            xl = wk.tile([128, 4 * S], bf16, tag="xl")
            nc.sync.dma_start(xl[:], dram[f"{p}_xlast"].ap())

            # ---- a_dst at last nodes: mult+reduce, transpose, broadcast ----
            tmp4 = wk.tile([128, 4 * S], bf16, tag="tmp4")
            nc.vector.tensor_tensor(tmp4[:], xl[:], wdst[:], op=Alu.mult)
            adst = wk.tile([128, 4], f32, tag="adst")
            nc.vector.tensor_reduce(
                adst[:], tmp4[:].rearrange("p (j s) -> p j s", s=S),
                axis=AxX, op=Alu.add)
            tp = ps1.tile([4, 128], f32, tag="tp")
            nc.tensor.transpose(tp[:], adst[:], ident[:])
            adT = wk.tile([4, 128], bf16, tag="adT")
            nc.vector.tensor_copy(adT[:], tp[:])
            adbc = ps1.tile([128, T], f32, tag="adbc")
            nc.tensor.matmul(adbc[:], Qm[:], adT[:], start=True, stop=True)

            # ---- a_src for all nodes: chunked mult + segmented reduce ----
            asrc = wk.tile([128, T], f32, tag="asrc")
            for c in range(NCHUNK):
                tmp = wk.tile([128, CH * SA], bf16, tag="tmp")
                nc.vector.tensor_tensor(
                    tmp[:], x[:, c * CH * SA:(c + 1) * CH * SA], wsrc[:],
                    op=Alu.mult)
                nc.vector.tensor_reduce(
                    asrc[:, c * CH:(c + 1) * CH],
                    tmp[:].rearrange("p (i s) -> p i s", s=SA),
                    axis=AxX, op=Alu.add)

            # ---- P = cnt * exp(leaky_relu(a_src + a_dst)) ----
            z = wk.tile([128, T], f32, tag="z")
            nc.vector.tensor_tensor(z[:], asrc[:], adbc[:], op=Alu.add)
            e = wk.tile([128, T], f32, tag="e")
            nc.vector.scalar_tensor_tensor(
                e[:], z[:], 0.2, z[:], op0=Alu.mult, op1=Alu.max)
            ex = wk.tile([128, T], f32, tag="ex")
            nc.scalar.activation(ex[:], e[:], Act.Exp)
            P = wk.tile([128, T], f32, tag="P")
            nc.vector.tensor_tensor(P[:], ex[:], cnt[:], op=Alu.mult)

            # ---- M[p, 4i+j] = P[p, i] * B[p, j] (block-diag weights) ----
            M = wk.tile([128, 4 * T], bf16, tag="M")
            Mv = M[:].rearrange("p (i j) -> p i j", j=4)
            for j in range(4):
                nc.vector.tensor_scalar(
                    Mv[:, :, j], P[:], Bm[:, j:j + 1], None, op0=Alu.mult)

            # ---- y^T aggregation: 128 small matmuls ----
            ynT = ps2.tile([128, 4 * T], f32, tag="ynT")
            for i in range(T):
                nc.tensor.matmul(
                    ynT[0:SA, 4 * i:4 * (i + 1)],
                    x[:, SA * i:SA * (i + 1)],
                    M[:, 4 * i:4 * (i + 1)],
                    start=True, stop=True)

            # ---- normalize by denominator (row 64 of y^T) ----
            ysb = wk.tile([S + 1, GPC], f32, tag="ysb")
            nc.scalar.copy(ysb[:], ynT[0:S + 1, :])
            dn = wk.tile([1, GPC], f32, tag="dn")
            nc.vector.tensor_scalar(
                dn[:], ysb[S:S + 1, :], eps[:], None, op0=Alu.add)
            rp = wk.tile([1, GPC], f32, tag="rp")
            nc.vector.reciprocal(rp[:], dn[:])
            rbc = ps1.tile([S, GPC], f32, tag="rbc")
            nc.tensor.matmul(rbc[:], ones64[:], rp[:], start=True, stop=True)
            ynrm = wk.tile([S, GPC], bf16, tag="ynrm")
            nc.vector.tensor_tensor(ynrm[:], ysb[0:S, :], rbc[:], op=Alu.mult)

            # ---- project + bias + sigmoid ----
            hT = ps1.tile([H, GPC], f32, tag="hT")
            nc.tensor.matmul(hT[:], Wb[:], ynrm[:], start=True, stop=True)
            sg = wk.tile([H, GPC], bf16, tag="sig")
            nc.scalar.activation(sg[:], hT[:], Act.Sigmoid, bias=bias[:])
            sig[p] = sg

        # ---- combine branches + MLP head ----
        prod = wk.tile([H, GPC], bf16, tag="prod")
        nc.vector.tensor_tensor(prod[:], sig["u"][:], sig["d"][:], op=Alu.mult)
        o_ps = ps1.tile([1, GPC], f32, tag="o_ps")
        nc.tensor.matmul(o_ps[:], mlpW[:], prod[:], start=True, stop=True)
        o_sb = wk.tile([1, GPC], f32, tag="o_sb")
        nc.vector.tensor_scalar(
            o_sb[:], o_ps[:], mlpb[:], None, op0=Alu.add)
        nc.sync.dma_start(out_dram.ap(), o_sb[:])

    nc.compile()
    return nc


def _get_module():
    if "nc" not in _CACHE:
        _CACHE["nc"] = _build_module()
    return _CACHE["nc"]


def _prep_branch(x, ei, W, att_src, att_dst, bias):
    """Host-side sharding + graph-format prep for one branch."""
    import ml_dtypes
    bf = ml_dtypes.bfloat16
    x = np.asarray(x, np.float32)
    src = np.asarray(ei[0]).astype(np.int64)
    dst = np.asarray(ei[1]).astype(np.int64)
    W = np.asarray(W, np.float32)
    w_src = (W @ np.asarray(att_src, np.float32)).astype(np.float32)
    w_dst = (W @ np.asarray(att_dst, np.float32)).astype(np.float32)

    valid = (dst % NPG) == (NPG - 1)
    cnt = np.bincount(src[valid], minlength=N).astype(np.float32)

    per_core = []
    for c in range(NC):
        xs = x[c * NPC:(c + 1) * NPC]
        xab = np.zeros((T, 128, SA), np.float32)
        xab[:, :, :S] = xs.reshape(T, 128, S)
        xab[:, :, S] = 1.0
        xab = np.ascontiguousarray(
            xab.transpose(1, 0, 2).reshape(128, T * SA)).astype(bf)
        cnt_t = np.ascontiguousarray(
            cnt[c * NPC:(c + 1) * NPC].reshape(T, 128).T)
        xlast = np.ascontiguousarray(
            xs[NPG - 1::NPG].reshape(128, 4 * S)).astype(bf)
        per_core.append({"xab": xab, "cnt": cnt_t, "xlast": xlast})

    wsrc_rep = np.zeros((128, CH * SA), np.float32)
    wsrc_rep.reshape(128, CH, SA)[:, :, :S] = w_src
    wdst_rep = np.broadcast_to(w_dst, (128, 4, S)).reshape(128, 4 * S)
    shared = {
        "wsrc": wsrc_rep.astype(bf),
        "wdst": wdst_rep.astype(bf),
        "W": W.astype(bf),
        "bias": np.asarray(bias, np.float32).reshape(H, 1),
    }
    return per_core, shared


def _build_in_maps(inputs):
    import ml_dtypes
    bf = ml_dtypes.bfloat16
    pcs = {}
    shareds = {}
    pcs["u"], shareds["u"] = _prep_branch(
        inputs["up_x"], inputs["up_edge_index"], inputs["up_W"],
        inputs["up_att_src"], inputs["up_att_dst"], inputs["up_bias"])
    pcs["d"], shareds["d"] = _prep_branch(
        inputs["down_x"], inputs["down_edge_index"], inputs["down_W"],
        inputs["down_att_src"], inputs["down_att_dst"], inputs["down_bias"])

    pp = np.arange(128)
    Qm = np.zeros((4, 128), np.float32)
    Qm[pp // 32, pp] = 1.0
    Bm = np.zeros((128, 4), np.float32)
    Bm[pp, pp // 32] = 1.0

    common = {
        "Qm": Qm.astype(bf),
        "Bm": Bm,
        "ones64": np.ones((1, S), np.float32),
        "ident": np.eye(128, dtype=np.float32),
        "mlpW": np.asarray(inputs["mlp_W"], np.float32).reshape(H, 1).astype(bf),
        "mlpb": np.asarray(inputs["mlp_b"], np.float32).reshape(1, 1),
        "eps": np.full((1, 1), 1e-16, np.float32),
    }
    for p in ("u", "d"):
        for k, v in shareds[p].items():
            common[f"{p}_{k}"] = v

    in_maps = []
    for c in range(NC):
        m = dict(common)
        for p in ("u", "d"):
            for k, v in pcs[p][c].items():
                m[f"{p}_{k}"] = v
        in_maps.append(m)
    return in_maps


def kernel(**inputs):
    from concourse.bass_utils import run_bass_kernel_spmd

    nc = _get_module()
    in_maps = _build_in_maps(inputs)
    res = run_bass_kernel_spmd(nc, in_maps, core_ids=list(range(NC)))
    out = np.concatenate(
        [np.asarray(r["out"], np.float32).reshape(GPC) for r in res.results])
    return out.reshape(G, 1)
